# revision 19
# baseline (speedup 1.0000x reference)
"""2-layer GCN + JumpingKnowledge(cat) + Linear on 8 Trainium2 NeuronCores.

Strategy (graph-parallel, nodes sharded 6250/core):
  - g = dinv * (x @ W) computed per-core (TensorE + DVE), written to HBM,
    AllGather'd (bf16) so every core holds the full node-feature table.
  - Message passing per destination PAIR of tiles (256 dsts): dma_gather
    pulls the unique source rows for the pair; aggregation is a chain of
    TensorE matmuls  psum[feat, 256 dst] += M_chunk^T @ S_chunk  where S
    (host-built) carries the symmetric-norm coefficients dinv[dst]
    (multiplicity-summed, sources dedup'd ACROSS the pair - the pair-level
    dedup cuts ~12% of gathered rows vs per-tile groups).  Self-loops use
    the local g tile against diag(dinv).
  - relu(+bias) on ScalarE writes the transposed activations x^T directly,
    which feeds the next layer's matmuls without any transposes.
  - Final: out = x1 @ lin_W[:D] + x2 @ lin_W[D:] (+ lin_b) per tile.

The dma_gather descriptor-generation core (single Q7 pair, ~8ns/row) is the
critical path; everything else hides under it.  To keep it busy during the
AllGather latency windows, edges whose SOURCE is core-local for the first
LOCAL_PAIRS pairs are gathered from the core's own pre-collective table
(gcat, written right after the g-phase matmuls) in a pass-0 that runs while
the AllGather is in flight.  Remote-sourced edges follow in pass-1 (block
A) / pass-2 (block B).  Pass-2 runs in REVERSE pair order so the layer-2
block-B table can be produced (and its AllGather launched) mid-stream,
eliminating the layer-transition stall.

dma_gather needs int16 indices, so the global table is built as two block
tables (each core contributes rows [0,3200) to table A and [3200,6272) to
table B); the local table gcat is the core's own 6272 rows in node order.
"""
import numpy as np
import ml_dtypes

import concourse.bass as bass
import concourse.bacc as bacc
import concourse.mybir as mybir
import concourse.tile as tile
from concourse._compat import get_trn_type
from concourse.bass_utils import run_bass_kernel_spmd
from concourse.library_config import mlp
from concourse.masks import make_identity

P = 128
W2P = 256  # dst width of one pair group
N_CORES = 8
LOCAL_PAIRS = 15  # pairs whose local-source edges go through pass-0

f32 = mybir.dt.float32
bf16 = mybir.dt.bfloat16
i16 = mybir.dt.int16
TABLE_DT = bf16
TABLE_NP = np.float32 if TABLE_DT == f32 else ml_dtypes.bfloat16


def _pack_idx(u_pad, nch):
    """int16 rows wrapped (nch*8, 16).T and tiled to 128 partitions."""
    wrapped = u_pad.astype(np.int16).reshape(nch * 8, 16).T
    return np.tile(wrapped, (8, 1))


def _preprocess(x, edge_index):
    """Host-side (numpy): shard, block-split, local/remote split, pair-level
    dedup, gather indices and the norm-coefficient matrices S."""
    N, D = x.shape
    assert D == P
    E = edge_index.shape[1]
    C = N_CORES
    NPC = (N + C - 1) // C
    assert NPC * C == N, "node count must split evenly across cores"
    NPAD = ((NPC + P - 1) // P) * P
    T = NPAD // P
    TA = (T + 1) // 2
    TB = T - TA
    BLKA, BLKB = TA * P, TB * P
    FULLA, FULLB = C * BLKA, C * BLKB
    assert FULLA <= 32768 and FULLB <= 32768 and NPAD <= 32768
    NP_ = (T + 1) // 2  # number of pairs
    pairs = [tuple(range(t, min(t + 2, T))) for t in range(0, T, 2)]
    assert len(pairs) == NP_

    src = edge_index[0].astype(np.int64)
    dst = edge_index[1].astype(np.int64)

    deg = np.bincount(dst, minlength=N).astype(np.float32) + 1.0
    dinv = (1.0 / np.sqrt(deg)).astype(np.float32)

    score = src // NPC
    off = src % NPC
    blk = (off >= BLKA).astype(np.int64)
    row_in_blk = np.where(blk == 0, score * BLKA + off,
                          score * BLKB + (off - BLKA))
    core = dst // NPC
    dloc = dst % NPC
    pidx = dloc // W2P          # pair index
    dcol = dloc % W2P           # column within the pair's 256-wide S
    vals = dinv[dst]

    is_local = (score == core) & (pidx < LOCAL_PAIRS)

    # ---- remote edges: grouped by (core, pair, half) -------------------
    rm = ~is_local
    gkey_r = (core[rm] * NP_ + pidx[rm]) * 2 + blk[rm]
    order_r = np.lexsort((row_in_blk[rm], gkey_r))
    gkey_rs = gkey_r[order_r]
    rows_rs = row_in_blk[rm][order_r]
    dcol_rs = dcol[rm][order_r]
    vals_rs = vals[rm][order_r]
    n_groups_r = C * NP_ * 2
    bounds_r = np.searchsorted(gkey_rs, np.arange(n_groups_r + 1))

    uniq_r = [None] * n_groups_r
    invs_r = [None] * n_groups_r
    for g in range(n_groups_r):
        lo, hi = bounds_r[g], bounds_r[g + 1]
        r = rows_rs[lo:hi]
        u = np.unique(r)
        uniq_r[g] = u
        invs_r[g] = np.searchsorted(u, r)

    # ---- local edges: grouped by (core, pair); index into gcat ---------
    row_loc = off
    gkey_l = core[is_local] * LOCAL_PAIRS + pidx[is_local]
    order_l = np.lexsort((row_loc[is_local], gkey_l))
    gkey_ls = gkey_l[order_l]
    rows_ls = row_loc[is_local][order_l]
    dcol_ls = dcol[is_local][order_l]
    vals_ls = vals[is_local][order_l]
    n_groups_l = C * LOCAL_PAIRS
    bounds_l = np.searchsorted(gkey_ls, np.arange(n_groups_l + 1))

    uniq_l = [None] * n_groups_l
    invs_l = [None] * n_groups_l
    for g in range(n_groups_l):
        lo, hi = bounds_l[g], bounds_l[g + 1]
        r = rows_ls[lo:hi]
        u = np.unique(r)
        uniq_l[g] = u
        invs_l[g] = np.searchsorted(u, r)

    # SPMD-uniform schedules (max over cores)
    nuniq_r = np.zeros((C, NP_, 2), np.int64)
    for g in range(n_groups_r):
        c, rem = divmod(g, NP_ * 2)
        p, h = divmod(rem, 2)
        nuniq_r[c, p, h] = len(uniq_r[g])
    sched_r = (np.maximum(nuniq_r.max(axis=0), 1) + P - 1) // P  # [NP_, 2]
    SCH = int(sched_r.sum())

    nuniq_l = np.zeros((C, LOCAL_PAIRS), np.int64)
    for g in range(n_groups_l):
        c, p = divmod(g, LOCAL_PAIRS)
        nuniq_l[c, p] = len(uniq_l[g])
    sched_l = (np.maximum(nuniq_l.max(axis=0), 1) + P - 1) // P  # [LOCAL_PAIRS]
    SCHL = int(sched_l.sum())

    # quad groups: one gather per (2 pairs, half); chunks laid out
    # (quad, half, pair-in-quad) so each gather's chunks are contiguous
    quads = [tuple(range(p, min(p + 2, NP_))) for p in range(0, NP_, 2)]
    chunk_off_r = np.zeros((NP_, 2), np.int64)
    acc = 0
    for qd in quads:
        for h in range(2):
            for p in qd:
                chunk_off_r[p, h] = acc
                acc += int(sched_r[p, h])
    assert acc == SCH
    lquads = [tuple(p for p in qd if p < LOCAL_PAIRS) for qd in quads]
    lquads = [qd for qd in lquads if qd]
    chunk_off_l = np.zeros(LOCAL_PAIRS, np.int64)
    acc = 0
    for qd in lquads:
        for p in qd:
            chunk_off_l[p] = acc
            acc += int(sched_l[p])
    assert acc == SCHL

    per_core = []
    for c in range(C):
        idx_r = np.zeros((P, SCH * 8), np.int16)
        idx_l = np.zeros((P, SCHL * 8), np.int16)

        sr = np.zeros((SCH, P, W2P), np.float32)
        sl = np.zeros((SCHL, P, W2P), np.float32)

        for p in range(NP_):
            for h in range(2):
                g = (c * NP_ + p) * 2 + h
                nch = int(sched_r[p, h])
                K = nch * P
                u = uniq_r[g]
                u_pad = np.zeros(K, np.int64)
                u_pad[: len(u)] = u
                co = int(chunk_off_r[p, h])
                idx_r[:, co * 8: (co + nch) * 8] = _pack_idx(u_pad, nch)
                lo, hi = bounds_r[g], bounds_r[g + 1]
                slot = invs_r[g]
                np.add.at(sr, (co + slot // P, slot % P, dcol_rs[lo:hi]),
                          vals_rs[lo:hi])
        for p in range(LOCAL_PAIRS):
            g = c * LOCAL_PAIRS + p
            nch = int(sched_l[p])
            K = nch * P
            u = uniq_l[g]
            u_pad = np.zeros(K, np.int64)
            u_pad[: len(u)] = u
            co = int(chunk_off_l[p])
            idx_l[:, co * 8: (co + nch) * 8] = _pack_idx(u_pad, nch)
            lo, hi = bounds_l[g], bounds_l[g + 1]
            slot = invs_l[g]
            np.add.at(sl, (co + slot // P, slot % P, dcol_ls[lo:hi]),
                      vals_ls[lo:hi])

        smat_r = np.ascontiguousarray(sr.transpose(1, 0, 2)).reshape(P, SCH * W2P)
        smat_l = np.ascontiguousarray(sl.transpose(1, 0, 2)).reshape(P, SCHL * W2P)

        xT = np.zeros((P, NPAD), np.float32)
        xT[:, :NPC] = x[c * NPC: (c + 1) * NPC].T
        dv = np.zeros(NPAD, np.float32)
        dv[:NPC] = dinv[c * NPC: (c + 1) * NPC]
        dinv_tiles = np.ascontiguousarray(dv.reshape(T, P).T)  # [P, T]

        per_core.append({"xT": xT, "dinv": dinv_tiles,
                         "idx": idx_r, "smat": smat_r,
                         "idxl": idx_l, "smatl": smat_l})

    plan = {
        "N": N, "D": D, "E": E, "C": C, "NPC": NPC, "NPAD": NPAD, "T": T,
        "TA": TA, "TB": TB, "BLKA": BLKA, "BLKB": BLKB,
        "FULLA": FULLA, "FULLB": FULLB, "SCH": SCH, "SCHL": SCHL,
        "NP": NP_, "sched": sched_r, "chunk_off": chunk_off_r,
        "sched_l": sched_l, "chunk_off_l": chunk_off_l, "pairs": pairs,
        "quads": quads, "lquads": lquads,
    }
    return plan, per_core


def _build(plan):
    T, TA, TB = plan["T"], plan["TA"], plan["TB"]
    NPAD = plan["NPAD"]
    BLKA, BLKB = plan["BLKA"], plan["BLKB"]
    FULLA, FULLB = plan["FULLA"], plan["FULLB"]
    SCH, SCHL = plan["SCH"], plan["SCHL"]
    sched = plan["sched"]
    chunk_off = plan["chunk_off"]
    sched_l = plan["sched_l"]
    chunk_off_l = plan["chunk_off_l"]
    plan_pairs = plan["pairs"]
    quads = plan["quads"]
    lquads = plan["lquads"]
    # max chunks in one (quad, half) gather
    maxch = max(sum(int(sched[p, h]) for p in qd) for qd in quads for h in range(2))
    maxch = max(maxch, max(sum(int(sched_l[p]) for p in qd) for qd in lquads))

    nc = bacc.Bacc(
        get_trn_type() or "TRN2",
        target_bir_lowering=False,
        debug=False,
        num_devices=N_CORES,
    )
    xT_in = nc.dram_tensor("xT", [P, NPAD], f32, kind="ExternalInput").ap()
    w1_in = nc.dram_tensor("w1", [P, P], f32, kind="ExternalInput").ap()
    w2_in = nc.dram_tensor("w2", [P, P], TABLE_DT, kind="ExternalInput").ap()
    lin1_in = nc.dram_tensor("lin1", [P, P], TABLE_DT, kind="ExternalInput").ap()
    lin2_in = nc.dram_tensor("lin2", [P, P], TABLE_DT, kind="ExternalInput").ap()
    b1_in = nc.dram_tensor("b1", [P, 1], f32, kind="ExternalInput").ap()
    b2_in = nc.dram_tensor("b2", [P, 1], f32, kind="ExternalInput").ap()
    linb_in = nc.dram_tensor("linb", [P, P], f32, kind="ExternalInput").ap()
    dinv_in = nc.dram_tensor("dinv", [P, T], f32, kind="ExternalInput").ap()
    idx_in = nc.dram_tensor("idx", [P, SCH * 8], i16, kind="ExternalInput").ap()
    smat_in = nc.dram_tensor("smat", [P, SCH * W2P], TABLE_DT, kind="ExternalInput").ap()
    idxl_in = nc.dram_tensor("idxl", [P, SCHL * 8], i16, kind="ExternalInput").ap()
    smatl_in = nc.dram_tensor("smatl", [P, SCHL * W2P], TABLE_DT, kind="ExternalInput").ap()
    out_ap = nc.dram_tensor("out", [NPAD, P], f32, kind="ExternalOutput").ap()
    out_v = out_ap.rearrange("(t p) f -> p t f", p=P)

    nc.gpsimd.load_library(mlp)

    with tile.TileContext(nc) as tc:
        with (
            tc.tile_pool(name="dram", bufs=1, space="DRAM") as dram,
            tc.tile_pool(name="consts", bufs=1) as consts,
            tc.tile_pool(name="xTp", bufs=1) as xTp,
            tc.tile_pool(name="stages", bufs=1) as stages,
            tc.tile_pool(name="msg", bufs=3) as msgp,
            tc.tile_pool(name="smat", bufs=3) as smatp,
            tc.tile_pool(name="diag", bufs=3) as diagp,
            tc.tile_pool(name="otile", bufs=3) as otilep,
            tc.tile_pool(name="ps_phase", bufs=3, space="PSUM") as psphase,
            tc.tile_pool(name="ps_agg", bufs=4, space="PSUM") as psagg,
        ):
            xT = xTp.tile([P, NPAD], f32)
            nc.sync.dma_start(xT[:], xT_in[:])
            w1 = consts.tile([P, P], f32, tag="w1")
            nc.sync.dma_start(w1[:], w1_in[:])
            w2 = consts.tile([P, P], TABLE_DT, tag="w2")
            nc.sync.dma_start(w2[:], w2_in[:])
            lin1 = consts.tile([P, P], TABLE_DT, tag="lin1")
            nc.sync.dma_start(lin1[:], lin1_in[:])
            lin2 = consts.tile([P, P], TABLE_DT, tag="lin2")
            nc.sync.dma_start(lin2[:], lin2_in[:])
            b1 = consts.tile([P, 1], f32, tag="b1")
            nc.sync.dma_start(b1[:], b1_in[:])
            b2 = consts.tile([P, 1], f32, tag="b2")
            nc.sync.dma_start(b2[:], b2_in[:])
            linb = consts.tile([P, P], f32, tag="linb")
            nc.sync.dma_start(linb[:], linb_in[:])
            dinv = consts.tile([P, T], f32, tag="dinv")
            nc.sync.dma_start(dinv[:], dinv_in[:])
            idx_sb = consts.tile([P, SCH * 8], i16, tag="idx")
            nc.sync.dma_start(idx_sb[:], idx_in[:])
            idxl_sb = consts.tile([P, SCHL * 8], i16, tag="idxl")
            nc.sync.dma_start(idxl_sb[:], idxl_in[:])
            ident = consts.tile([P, P], f32, tag="ident")
            make_identity(nc, ident[:])

            def blk_pair(tag, dt=f32):
                a = stages.tile([P, BLKA], dt, tag=f"{tag}A", name=f"{tag}A")
                b = stages.tile([P, BLKB], dt, tag=f"{tag}B", name=f"{tag}B")
                return [a, b]

            gstage = blk_pair("gstage", TABLE_DT)
            x1T = blk_pair("x1T", TABLE_DT)
            x2T = blk_pair("x2T", TABLE_DT)

            g_loc = [[None, None], [None, None]]
            g_full = [[None, None], [None, None]]
            gcat = [None, None]
            for layer in range(2):
                for h, (blkrows, fullrows) in enumerate([(BLKA, FULLA), (BLKB, FULLB)]):
                    g_loc[layer][h] = dram.tile(
                        [blkrows, P], TABLE_DT, tag=f"gloc{layer}{h}", name=f"gloc{layer}{h}"
                    )
                    g_full[layer][h] = dram.tile(
                        [fullrows, P], TABLE_DT, tag=f"gfull{layer}{h}", name=f"gfull{layer}{h}"
                    )
                gcat[layer] = dram.tile(
                    [NPAD, P], TABLE_DT, tag=f"gcat{layer}", name=f"gcat{layer}"
                )

            def loc_tile(t):
                return (0, t) if t < TA else (1, t - TA)

            def phase_g_block(src_stages, w_tile, layer, h):
                t0 = 0 if h == 0 else TA
                nt = TA if h == 0 else TB
                gs = gstage[h]
                for i in range(nt):
                    t = t0 + i
                    hh, ii = loc_tile(t)
                    ps = psphase.tile([P, P], f32, tag="ps_phase", name="psph")
                    nc.tensor.matmul(
                        ps[:], lhsT=src_stages[hh][:, bass.ts(ii, P)], rhs=w_tile[:],
                        start=True, stop=True,
                    )
                    nc.vector.tensor_scalar(
                        gs[:, bass.ts(i, P)], ps[:],
                        dinv[:, t: t + 1], None, mybir.AluOpType.mult,
                    )
                gl = g_loc[layer][h]
                nc.sync.dma_start(gl[:].rearrange("(t p) f -> p t f", p=P), gs[:])
                r0 = 0 if h == 0 else BLKA
                gc = gcat[layer][r0: r0 + nt * P]
                nc.sync.dma_start(gc.rearrange("(t p) f -> p t f", p=P), gs[:])
                nc.gpsimd.collective_compute(
                    "AllGather",
                    mybir.AluOpType.bypass,
                    replica_groups=[list(range(N_CORES))],
                    ins=[gl.opt()],
                    outs=[g_full[layer][h].opt()],
                )

            # partial padded to a multiple of 256 so every pair has a full
            # 256-wide slice (last pair's upper half is scratch)
            NPADW = ((NPAD + W2P - 1) // W2P) * W2P
            partial = xTp.tile([P, NPADW], f32, tag="partial", name="partial")
            pairs = plan_pairs

            def pair_slice(pr):
                c0 = pr[0] * P
                return partial[:, c0: c0 + W2P]

            def agg_chunks(msg, s_sb, tot):
                ps = psagg.tile([P, W2P], f32, tag="ps_agg", name="psagg")
                for j in range(tot):
                    nc.tensor.matmul(
                        ps[:], lhsT=msg[:, j, :],
                        rhs=s_sb[:, j * W2P: (j + 1) * W2P],
                        start=(j == 0), stop=(j == tot - 1),
                    )
                return ps

            def agg_chunks_at(msg, s_sb, jo, tot):
                """Matmul chunks [jo, jo+tot) of a quad gather against the
                matching S columns of its s_sb."""
                ps = psagg.tile([P, W2P], f32, tag="ps_agg", name="psagg")
                for j in range(tot):
                    nc.tensor.matmul(
                        ps[:], lhsT=msg[:, jo + j, :],
                        rhs=s_sb[:, (jo + j) * W2P: (jo + j + 1) * W2P],
                        start=(j == 0), stop=(j == tot - 1),
                    )
                return ps

            def pass_local(layer):
                """Gather local-source messages from gcat (no collective
                dependency) -> partial (first LOCAL_PAIRS pairs)."""
                for qd in lquads:
                    tots = [int(sched_l[p]) for p in qd]
                    tot = sum(tots)
                    K = tot * P
                    co = int(chunk_off_l[qd[0]])
                    msg = msgp.tile([P, maxch, P], TABLE_DT, tag="msg", name="msg")
                    s_sb = smatp.tile([P, maxch * W2P], TABLE_DT, tag="smat", name="ssb")
                    nc.sync.dma_start(
                        s_sb[:, 0: tot * W2P], smatl_in[:, co * W2P: (co + tot) * W2P]
                    )
                    nc.gpsimd.dma_gather(
                        msg[:, 0:tot, :],
                        gcat[layer][:],
                        idxl_sb[:, co * 8: (co + tot) * 8],
                        K, K, P,
                        single_packet=False,
                    )
                    jo = 0
                    for p, tp in zip(qd, tots):
                        ps = agg_chunks_at(msg, s_sb, jo, tp)
                        jo += tp
                        nc.vector.tensor_copy(out=pair_slice(pairs[p]), in_=ps[:])

            def layer_agg(layer, xout, bias, hooks=None, post_tile=None):
                # pass 1: remote block-A chunks -> partial (+= for pairs that
                # had a local pass-0, copy otherwise); one gather per quad
                for qd in quads:
                    tots = [int(sched[p, 0]) for p in qd]
                    tot = sum(tots)
                    K = tot * P
                    co = int(chunk_off[qd[0], 0])
                    msg = msgp.tile([P, maxch, P], TABLE_DT, tag="msg", name="msg")
                    s_sb = smatp.tile([P, maxch * W2P], TABLE_DT, tag="smat", name="ssb")
                    nc.sync.dma_start(
                        s_sb[:, 0: tot * W2P], smat_in[:, co * W2P: (co + tot) * W2P]
                    )
                    nc.gpsimd.dma_gather(
                        msg[:, 0:tot, :],
                        g_full[layer][0][:],
                        idx_sb[:, co * 8: (co + tot) * 8],
                        K, K, P,
                        single_packet=False,
                    )
                    jo = 0
                    for p, tp in zip(qd, tots):
                        ps = agg_chunks_at(msg, s_sb, jo, tp)
                        jo += tp
                        if p < LOCAL_PAIRS:
                            nc.vector.tensor_tensor(
                                out=pair_slice(pairs[p]), in0=pair_slice(pairs[p]),
                                in1=ps[:], op=mybir.AluOpType.add,
                            )
                        else:
                            nc.vector.tensor_copy(out=pair_slice(pairs[p]), in_=ps[:])
                # pass 2 (REVERSED quad order): remote block-B + self-loops,
                # add partial, relu
                for rq, qd in enumerate(reversed(quads)):
                    if hooks and rq in hooks:
                        hooks[rq]()
                    tots = [int(sched[p, 1]) for p in qd]
                    tot = sum(tots)
                    K = tot * P
                    co = int(chunk_off[qd[0], 1])
                    msg = msgp.tile([P, maxch, P], TABLE_DT, tag="msg", name="msg")
                    s_sb = smatp.tile([P, maxch * W2P], TABLE_DT, tag="smat", name="ssb")
                    nc.sync.dma_start(
                        s_sb[:, 0: tot * W2P], smat_in[:, co * W2P: (co + tot) * W2P]
                    )
                    nc.gpsimd.dma_gather(
                        msg[:, 0:tot, :],
                        g_full[layer][1][:],
                        idx_sb[:, co * 8: (co + tot) * 8],
                        K, K, P,
                        single_packet=False,
                    )
                    jo = 0
                    for p, tp in zip(qd, tots):
                        pr = pairs[p]
                        ps = psagg.tile([P, W2P], f32, tag="ps_agg", name="psagg")
                        for j in range(tp):
                            nc.tensor.matmul(
                                ps[:], lhsT=msg[:, jo + j, :],
                                rhs=s_sb[:, (jo + j) * W2P: (jo + j + 1) * W2P],
                                start=(j == 0), stop=False,
                            )
                        jo += tp
                        for k, t in enumerate(pr):
                            diag = diagp.tile([P, P], TABLE_DT, tag="diag", name="diag")
                            nc.vector.tensor_scalar(
                                diag[:], ident[:], dinv[:, t: t + 1], None,
                                mybir.AluOpType.mult,
                            )
                            hh, ii = loc_tile(t)
                            nc.tensor.matmul(
                                ps[:, k * P: (k + 1) * P],
                                lhsT=gstage[hh][:, bass.ts(ii, P)], rhs=diag[:],
                                start=(tp == 0), stop=True,
                            )
                        if len(pr) == 1 and tp > 0:
                            # close the accumulation on the unused upper half
                            nc.tensor.matmul(
                                ps[:, P:W2P], lhsT=msg[:, jo - 1, :],
                                rhs=s_sb[:, (jo - 1) * W2P + P: jo * W2P],
                                start=False, stop=True,
                            )
                        nc.vector.tensor_tensor(
                            out=ps[:], in0=ps[:], in1=pair_slice(pr),
                            op=mybir.AluOpType.add,
                        )
                        for k, t in enumerate(pr):
                            hh, ii = loc_tile(t)
                            nc.scalar.activation(
                                xout[hh][:, bass.ts(ii, P)], ps[:, k * P: (k + 1) * P],
                                mybir.ActivationFunctionType.Relu, bias=bias[:],
                            )
                            if post_tile is not None:
                                post_tile(t)

            xT_stages = [xT[:, 0:BLKA], xT[:, BLKA:NPAD]]

            class _W:
                def __init__(self, aps):
                    self.aps = aps

                def __getitem__(self, h):
                    return self.aps[h]

            phase_g_block(_W(xT_stages), w1, 0, 0)
            phase_g_block(_W(xT_stages), w1, 0, 1)
            pass_local(0)
            # B-half tiles (25..48) are pairs 12..24 = reversed quads 0..6;
            # hook the layer-2 B-table production right after them.
            layer_agg(
                0, x1T, b1,
                hooks={7: lambda: phase_g_block(x1T, w2, 1, 1)},
            )
            phase_g_block(x1T, w2, 1, 0)
            pass_local(1)

            def final_tile(t):
                hh, ii = loc_tile(t)
                ps = psphase.tile([P, P], f32, tag="ps_phase", name="psph")
                nc.tensor.matmul(
                    ps[:], lhsT=x1T[hh][:, bass.ts(ii, P)], rhs=lin1[:],
                    start=True, stop=False,
                )
                nc.tensor.matmul(
                    ps[:], lhsT=x2T[hh][:, bass.ts(ii, P)], rhs=lin2[:],
                    start=False, stop=True,
                )
                ot = otilep.tile([P, P], f32, tag="otile", name="otile")
                nc.vector.tensor_tensor(
                    out=ot[:], in0=ps[:], in1=linb[:], op=mybir.AluOpType.add
                )
                nc.sync.dma_start(out_v[:, t, :], ot[:])

            layer_agg(1, x2T, b2, post_tile=final_tile)

    nc.compile()
    return nc


def _in_maps(plan, per_core, W1, b1, W2, b2, lin_W, lin_b):
    D = plan["D"]
    maps = []
    for c in range(plan["C"]):
        pc = per_core[c]
        maps.append(
            {
                "xT": pc["xT"],
                "w1": np.asarray(W1, np.float32),
                "w2": np.asarray(W2, np.float32).astype(TABLE_NP),
                "lin1": np.ascontiguousarray(np.asarray(lin_W, np.float32)[:D]).astype(TABLE_NP),
                "lin2": np.ascontiguousarray(np.asarray(lin_W, np.float32)[D:]).astype(TABLE_NP),
                "b1": np.asarray(b1, np.float32)[:, None],
                "b2": np.asarray(b2, np.float32)[:, None],
                "linb": np.tile(np.asarray(lin_b, np.float32), (P, 1)),
                "dinv": pc["dinv"],
                "idx": pc["idx"],
                "smat": pc["smat"].astype(TABLE_NP),
                "idxl": pc["idxl"],
                "smatl": pc["smatl"].astype(TABLE_NP),
            }
        )
    return maps


def kernel(x, edge_index, W1, b1, W2, b2, lin_W, lin_b):
    x = np.asarray(x, np.float32)
    edge_index = np.asarray(edge_index)

    plan, per_core = _preprocess(x, edge_index)
    nc = _build(plan)
    maps = _in_maps(plan, per_core, W1, b1, W2, b2, lin_W, lin_b)

    N, D, C, NPC = plan["N"], plan["D"], plan["C"], plan["NPC"]
    last_err = None
    for _attempt in range(3):
        try:
            res = run_bass_kernel_spmd(nc, maps, list(range(C)))
            break
        except Exception as e:  # transient NRT device wedges happen
            last_err = e
    else:
        raise last_err

    out = np.empty((N, D), np.float32)
    for c in range(C):
        out[c * NPC: (c + 1) * NPC] = res.results[c]["out"][:NPC]
    return out


# revision 21
# speedup vs baseline: 1.0495x; 1.0495x over previous
"""2-layer GCN + JumpingKnowledge(cat) + Linear on 8 Trainium2 NeuronCores.

Strategy (graph-parallel, nodes sharded 6250/core):
  - g = dinv * (x @ W) computed per-core (TensorE + DVE), written to HBM,
    AllGather'd (bf16) so every core holds the full node-feature table.
  - Message passing per destination PAIR of tiles (256 dsts): dma_gather
    pulls the unique source rows for the pair; aggregation is a chain of
    TensorE matmuls  psum[feat, 256 dst] += M_chunk^T @ S_chunk  where S
    (host-built) carries the symmetric-norm coefficients dinv[dst]
    (multiplicity-summed, sources dedup'd ACROSS the pair - the pair-level
    dedup cuts ~12% of gathered rows vs per-tile groups).  Self-loops use
    the local g tile against diag(dinv).
  - relu(+bias) on ScalarE writes the transposed activations x^T directly,
    which feeds the next layer's matmuls without any transposes.
  - Final: out = x1 @ lin_W[:D] + x2 @ lin_W[D:] (+ lin_b) per tile.

The dma_gather descriptor-generation core (single Q7 pair, ~8ns/row) is the
critical path; everything else hides under it.  To keep it busy during the
AllGather latency windows, edges whose SOURCE is core-local for the first
LOCAL_PAIRS pairs are gathered from the core's own pre-collective table
(gcat, written right after the g-phase matmuls) in a pass-0 that runs while
the AllGather is in flight.  Remote-sourced edges follow in pass-1 (block
A) / pass-2 (block B).  Pass-2 runs in REVERSE pair order so the layer-2
block-B table can be produced (and its AllGather launched) mid-stream,
eliminating the layer-transition stall.

dma_gather needs int16 indices, so the global table is built as two block
tables (each core contributes rows [0,3200) to table A and [3200,6272) to
table B); the local table gcat is the core's own 6272 rows in node order.
"""
import numpy as np
import ml_dtypes

import concourse.bass as bass
import concourse.bacc as bacc
import concourse.mybir as mybir
import concourse.tile as tile
from concourse._compat import get_trn_type
from concourse.bass_utils import run_bass_kernel_spmd
from concourse.library_config import mlp
from concourse.masks import make_identity

P = 128
W2P = 256  # dst width of one pair group
N_CORES = 8
LOCAL_PAIRS = 25  # pairs whose local-source edges go through pass-0 (all)

f32 = mybir.dt.float32
bf16 = mybir.dt.bfloat16
i16 = mybir.dt.int16
TABLE_DT = bf16
TABLE_NP = np.float32 if TABLE_DT == f32 else ml_dtypes.bfloat16


def _pack_idx(u_pad, nch):
    """int16 rows wrapped (nch*8, 16).T and tiled to 128 partitions."""
    wrapped = u_pad.astype(np.int16).reshape(nch * 8, 16).T
    return np.tile(wrapped, (8, 1))


def _preprocess(x, edge_index):
    """Host-side (numpy): shard, block-split, local/remote split, pair-level
    dedup, gather indices and the norm-coefficient matrices S."""
    N, D = x.shape
    assert D == P
    E = edge_index.shape[1]
    C = N_CORES
    NPC = (N + C - 1) // C
    assert NPC * C == N, "node count must split evenly across cores"
    NPAD = ((NPC + P - 1) // P) * P
    T = NPAD // P
    TA = (T + 1) // 2
    TB = T - TA
    BLKA, BLKB = TA * P, TB * P
    FULLA, FULLB = C * BLKA, C * BLKB
    assert FULLA <= 32768 and FULLB <= 32768 and NPAD <= 32768
    NP_ = (T + 1) // 2  # number of pairs
    pairs = [tuple(range(t, min(t + 2, T))) for t in range(0, T, 2)]
    assert len(pairs) == NP_

    src = edge_index[0].astype(np.int64)
    dst = edge_index[1].astype(np.int64)

    deg = np.bincount(dst, minlength=N).astype(np.float32) + 1.0
    dinv = (1.0 / np.sqrt(deg)).astype(np.float32)

    score = src // NPC
    off = src % NPC
    blk = (off >= BLKA).astype(np.int64)
    row_in_blk = np.where(blk == 0, score * BLKA + off,
                          score * BLKB + (off - BLKA))
    core = dst // NPC
    dloc = dst % NPC
    pidx = dloc // W2P          # pair index
    dcol = dloc % W2P           # column within the pair's 256-wide S
    vals = dinv[dst]

    is_local = (score == core) & (pidx < LOCAL_PAIRS)

    # ---- remote edges: grouped by (core, pair, half) -------------------
    rm = ~is_local
    gkey_r = (core[rm] * NP_ + pidx[rm]) * 2 + blk[rm]
    order_r = np.lexsort((row_in_blk[rm], gkey_r))
    gkey_rs = gkey_r[order_r]
    rows_rs = row_in_blk[rm][order_r]
    dcol_rs = dcol[rm][order_r]
    vals_rs = vals[rm][order_r]
    n_groups_r = C * NP_ * 2
    bounds_r = np.searchsorted(gkey_rs, np.arange(n_groups_r + 1))

    uniq_r = [None] * n_groups_r
    invs_r = [None] * n_groups_r
    for g in range(n_groups_r):
        lo, hi = bounds_r[g], bounds_r[g + 1]
        r = rows_rs[lo:hi]
        u = np.unique(r)
        uniq_r[g] = u
        invs_r[g] = np.searchsorted(u, r)

    # ---- local edges: grouped by (core, pair); index into gcat ---------
    row_loc = off
    gkey_l = core[is_local] * LOCAL_PAIRS + pidx[is_local]
    order_l = np.lexsort((row_loc[is_local], gkey_l))
    gkey_ls = gkey_l[order_l]
    rows_ls = row_loc[is_local][order_l]
    dcol_ls = dcol[is_local][order_l]
    vals_ls = vals[is_local][order_l]
    n_groups_l = C * LOCAL_PAIRS
    bounds_l = np.searchsorted(gkey_ls, np.arange(n_groups_l + 1))

    uniq_l = [None] * n_groups_l
    invs_l = [None] * n_groups_l
    for g in range(n_groups_l):
        lo, hi = bounds_l[g], bounds_l[g + 1]
        r = rows_ls[lo:hi]
        u = np.unique(r)
        uniq_l[g] = u
        invs_l[g] = np.searchsorted(u, r)

    # SPMD-uniform schedules (max over cores)
    nuniq_r = np.zeros((C, NP_, 2), np.int64)
    for g in range(n_groups_r):
        c, rem = divmod(g, NP_ * 2)
        p, h = divmod(rem, 2)
        nuniq_r[c, p, h] = len(uniq_r[g])
    sched_r = (np.maximum(nuniq_r.max(axis=0), 1) + P - 1) // P  # [NP_, 2]
    SCH = int(sched_r.sum())

    nuniq_l = np.zeros((C, LOCAL_PAIRS), np.int64)
    for g in range(n_groups_l):
        c, p = divmod(g, LOCAL_PAIRS)
        nuniq_l[c, p] = len(uniq_l[g])
    sched_l = (np.maximum(nuniq_l.max(axis=0), 1) + P - 1) // P  # [LOCAL_PAIRS]
    SCHL = int(sched_l.sum())

    # quad groups: one gather per (2 pairs, half); chunks laid out
    # (quad, half, pair-in-quad) so each gather's chunks are contiguous
    quads = [tuple(range(p, min(p + 2, NP_))) for p in range(0, NP_, 2)]
    chunk_off_r = np.zeros((NP_, 2), np.int64)
    acc = 0
    for qd in quads:
        for h in range(2):
            for p in qd:
                chunk_off_r[p, h] = acc
                acc += int(sched_r[p, h])
    assert acc == SCH
    lquads = [tuple(p for p in qd if p < LOCAL_PAIRS) for qd in quads]
    lquads = [qd for qd in lquads if qd]
    chunk_off_l = np.zeros(LOCAL_PAIRS, np.int64)
    acc = 0
    for qd in lquads:
        for p in qd:
            chunk_off_l[p] = acc
            acc += int(sched_l[p])
    assert acc == SCHL

    per_core = []
    for c in range(C):
        idx_r = np.zeros((P, SCH * 8), np.int16)
        idx_l = np.zeros((P, SCHL * 8), np.int16)

        sr = np.zeros((SCH, P, W2P), np.float32)
        sl = np.zeros((SCHL, P, W2P), np.float32)

        for p in range(NP_):
            for h in range(2):
                g = (c * NP_ + p) * 2 + h
                nch = int(sched_r[p, h])
                K = nch * P
                u = uniq_r[g]
                u_pad = np.zeros(K, np.int64)
                u_pad[: len(u)] = u
                co = int(chunk_off_r[p, h])
                idx_r[:, co * 8: (co + nch) * 8] = _pack_idx(u_pad, nch)
                lo, hi = bounds_r[g], bounds_r[g + 1]
                slot = invs_r[g]
                np.add.at(sr, (co + slot // P, slot % P, dcol_rs[lo:hi]),
                          vals_rs[lo:hi])
        for p in range(LOCAL_PAIRS):
            g = c * LOCAL_PAIRS + p
            nch = int(sched_l[p])
            K = nch * P
            u = uniq_l[g]
            u_pad = np.zeros(K, np.int64)
            u_pad[: len(u)] = u
            co = int(chunk_off_l[p])
            idx_l[:, co * 8: (co + nch) * 8] = _pack_idx(u_pad, nch)
            lo, hi = bounds_l[g], bounds_l[g + 1]
            slot = invs_l[g]
            np.add.at(sl, (co + slot // P, slot % P, dcol_ls[lo:hi]),
                      vals_ls[lo:hi])

        smat_r = np.ascontiguousarray(sr.transpose(1, 0, 2)).reshape(P, SCH * W2P)
        smat_l = np.ascontiguousarray(sl.transpose(1, 0, 2)).reshape(P, SCHL * W2P)

        xT = np.zeros((P, NPAD), np.float32)
        xT[:, :NPC] = x[c * NPC: (c + 1) * NPC].T
        dv = np.zeros(NPAD, np.float32)
        dv[:NPC] = dinv[c * NPC: (c + 1) * NPC]
        dinv_tiles = np.ascontiguousarray(dv.reshape(T, P).T)  # [P, T]

        per_core.append({"xT": xT, "dinv": dinv_tiles,
                         "idx": idx_r, "smat": smat_r,
                         "idxl": idx_l, "smatl": smat_l})

    plan = {
        "N": N, "D": D, "E": E, "C": C, "NPC": NPC, "NPAD": NPAD, "T": T,
        "TA": TA, "TB": TB, "BLKA": BLKA, "BLKB": BLKB,
        "FULLA": FULLA, "FULLB": FULLB, "SCH": SCH, "SCHL": SCHL,
        "NP": NP_, "sched": sched_r, "chunk_off": chunk_off_r,
        "sched_l": sched_l, "chunk_off_l": chunk_off_l, "pairs": pairs,
        "quads": quads, "lquads": lquads,
    }
    return plan, per_core


def _build(plan):
    T, TA, TB = plan["T"], plan["TA"], plan["TB"]
    NPAD = plan["NPAD"]
    BLKA, BLKB = plan["BLKA"], plan["BLKB"]
    FULLA, FULLB = plan["FULLA"], plan["FULLB"]
    SCH, SCHL = plan["SCH"], plan["SCHL"]
    sched = plan["sched"]
    chunk_off = plan["chunk_off"]
    sched_l = plan["sched_l"]
    chunk_off_l = plan["chunk_off_l"]
    plan_pairs = plan["pairs"]
    quads = plan["quads"]
    lquads = plan["lquads"]
    # max chunks in one (quad, half) gather
    maxch = max(sum(int(sched[p, h]) for p in qd) for qd in quads for h in range(2))
    maxch = max(maxch, max(sum(int(sched_l[p]) for p in qd) for qd in lquads))

    nc = bacc.Bacc(
        get_trn_type() or "TRN2",
        target_bir_lowering=False,
        debug=False,
        num_devices=N_CORES,
    )
    xT_in = nc.dram_tensor("xT", [P, NPAD], f32, kind="ExternalInput").ap()
    w1_in = nc.dram_tensor("w1", [P, P], f32, kind="ExternalInput").ap()
    w2_in = nc.dram_tensor("w2", [P, P], TABLE_DT, kind="ExternalInput").ap()
    lin1_in = nc.dram_tensor("lin1", [P, P], TABLE_DT, kind="ExternalInput").ap()
    lin2_in = nc.dram_tensor("lin2", [P, P], TABLE_DT, kind="ExternalInput").ap()
    b1_in = nc.dram_tensor("b1", [P, 1], f32, kind="ExternalInput").ap()
    b2_in = nc.dram_tensor("b2", [P, 1], f32, kind="ExternalInput").ap()
    linb_in = nc.dram_tensor("linb", [P, P], f32, kind="ExternalInput").ap()
    dinv_in = nc.dram_tensor("dinv", [P, T], f32, kind="ExternalInput").ap()
    idx_in = nc.dram_tensor("idx", [P, SCH * 8], i16, kind="ExternalInput").ap()
    smat_in = nc.dram_tensor("smat", [P, SCH * W2P], TABLE_DT, kind="ExternalInput").ap()
    idxl_in = nc.dram_tensor("idxl", [P, SCHL * 8], i16, kind="ExternalInput").ap()
    smatl_in = nc.dram_tensor("smatl", [P, SCHL * W2P], TABLE_DT, kind="ExternalInput").ap()
    out_ap = nc.dram_tensor("out", [NPAD, P], f32, kind="ExternalOutput").ap()
    out_v = out_ap.rearrange("(t p) f -> p t f", p=P)

    nc.gpsimd.load_library(mlp)

    with tile.TileContext(nc) as tc:
        with (
            tc.tile_pool(name="dram", bufs=1, space="DRAM") as dram,
            tc.tile_pool(name="consts", bufs=1) as consts,
            tc.tile_pool(name="xTp", bufs=1) as xTp,
            tc.tile_pool(name="stages", bufs=1) as stages,
            tc.tile_pool(name="msg", bufs=4) as msgp,
            tc.tile_pool(name="smat", bufs=3) as smatp,
            tc.tile_pool(name="diag", bufs=3) as diagp,
            tc.tile_pool(name="otile", bufs=3) as otilep,
            tc.tile_pool(name="ps_phase", bufs=3, space="PSUM") as psphase,
            tc.tile_pool(name="ps_agg", bufs=4, space="PSUM") as psagg,
        ):
            xT = xTp.tile([P, NPAD], f32)
            nc.sync.dma_start(xT[:], xT_in[:])
            w1 = consts.tile([P, P], f32, tag="w1")
            nc.sync.dma_start(w1[:], w1_in[:])
            w2 = consts.tile([P, P], TABLE_DT, tag="w2")
            nc.sync.dma_start(w2[:], w2_in[:])
            lin1 = consts.tile([P, P], TABLE_DT, tag="lin1")
            nc.sync.dma_start(lin1[:], lin1_in[:])
            lin2 = consts.tile([P, P], TABLE_DT, tag="lin2")
            nc.sync.dma_start(lin2[:], lin2_in[:])
            b1 = consts.tile([P, 1], f32, tag="b1")
            nc.sync.dma_start(b1[:], b1_in[:])
            b2 = consts.tile([P, 1], f32, tag="b2")
            nc.sync.dma_start(b2[:], b2_in[:])
            linb = consts.tile([P, P], f32, tag="linb")
            nc.sync.dma_start(linb[:], linb_in[:])
            dinv = consts.tile([P, T], f32, tag="dinv")
            nc.sync.dma_start(dinv[:], dinv_in[:])
            idx_sb = consts.tile([P, SCH * 8], i16, tag="idx")
            nc.sync.dma_start(idx_sb[:], idx_in[:])
            idxl_sb = consts.tile([P, SCHL * 8], i16, tag="idxl")
            nc.sync.dma_start(idxl_sb[:], idxl_in[:])
            ident = consts.tile([P, P], f32, tag="ident")
            make_identity(nc, ident[:])

            def blk_pair(tag, dt=f32):
                a = stages.tile([P, BLKA], dt, tag=f"{tag}A", name=f"{tag}A")
                b = stages.tile([P, BLKB], dt, tag=f"{tag}B", name=f"{tag}B")
                return [a, b]

            gstage = blk_pair("gstage", TABLE_DT)
            x1T = blk_pair("x1T", TABLE_DT)
            x2T = blk_pair("x2T", TABLE_DT)

            g_loc = [[None, None], [None, None]]
            g_full = [[None, None], [None, None]]
            gcat = [None, None]
            for layer in range(2):
                for h, (blkrows, fullrows) in enumerate([(BLKA, FULLA), (BLKB, FULLB)]):
                    g_loc[layer][h] = dram.tile(
                        [blkrows, P], TABLE_DT, tag=f"gloc{layer}{h}", name=f"gloc{layer}{h}"
                    )
                    g_full[layer][h] = dram.tile(
                        [fullrows, P], TABLE_DT, tag=f"gfull{layer}{h}", name=f"gfull{layer}{h}"
                    )
                gcat[layer] = dram.tile(
                    [NPAD, P], TABLE_DT, tag=f"gcat{layer}", name=f"gcat{layer}"
                )

            def loc_tile(t):
                return (0, t) if t < TA else (1, t - TA)

            def phase_g_block(src_stages, w_tile, layer, h):
                t0 = 0 if h == 0 else TA
                nt = TA if h == 0 else TB
                gs = gstage[h]
                for i in range(nt):
                    t = t0 + i
                    hh, ii = loc_tile(t)
                    ps = psphase.tile([P, P], f32, tag="ps_phase", name="psph")
                    nc.tensor.matmul(
                        ps[:], lhsT=src_stages[hh][:, bass.ts(ii, P)], rhs=w_tile[:],
                        start=True, stop=True,
                    )
                    nc.vector.tensor_scalar(
                        gs[:, bass.ts(i, P)], ps[:],
                        dinv[:, t: t + 1], None, mybir.AluOpType.mult,
                    )
                gl = g_loc[layer][h]
                nc.sync.dma_start(gl[:].rearrange("(t p) f -> p t f", p=P), gs[:])
                r0 = 0 if h == 0 else BLKA
                gc = gcat[layer][r0: r0 + nt * P]
                nc.sync.dma_start(gc.rearrange("(t p) f -> p t f", p=P), gs[:])
                nc.gpsimd.collective_compute(
                    "AllGather",
                    mybir.AluOpType.bypass,
                    replica_groups=[list(range(N_CORES))],
                    ins=[gl.opt()],
                    outs=[g_full[layer][h].opt()],
                )

            # partial padded to a multiple of 256 so every pair has a full
            # 256-wide slice (last pair's upper half is scratch)
            NPADW = ((NPAD + W2P - 1) // W2P) * W2P
            partial = xTp.tile([P, NPADW], f32, tag="partial", name="partial")
            pairs = plan_pairs

            def pair_slice(pr):
                c0 = pr[0] * P
                return partial[:, c0: c0 + W2P]

            def agg_chunks(msg, s_sb, tot):
                ps = psagg.tile([P, W2P], f32, tag="ps_agg", name="psagg")
                for j in range(tot):
                    nc.tensor.matmul(
                        ps[:], lhsT=msg[:, j, :],
                        rhs=s_sb[:, j * W2P: (j + 1) * W2P],
                        start=(j == 0), stop=(j == tot - 1),
                    )
                return ps

            def agg_chunks_at(msg, s_sb, jo, tot):
                """Matmul chunks [jo, jo+tot) of a quad gather against the
                matching S columns of its s_sb."""
                ps = psagg.tile([P, W2P], f32, tag="ps_agg", name="psagg")
                for j in range(tot):
                    nc.tensor.matmul(
                        ps[:], lhsT=msg[:, jo + j, :],
                        rhs=s_sb[:, (jo + j) * W2P: (jo + j + 1) * W2P],
                        start=(j == 0), stop=(j == tot - 1),
                    )
                return ps

            def pass_local(layer):
                """Gather local-source messages from gcat (no collective
                dependency) -> partial (first LOCAL_PAIRS pairs)."""
                for qd in lquads:
                    tots = [int(sched_l[p]) for p in qd]
                    tot = sum(tots)
                    K = tot * P
                    co = int(chunk_off_l[qd[0]])
                    msg = msgp.tile([P, maxch, P], TABLE_DT, tag="msg", name="msg")
                    s_sb = smatp.tile([P, maxch * W2P], TABLE_DT, tag="smat", name="ssb")
                    nc.sync.dma_start(
                        s_sb[:, 0: tot * W2P], smatl_in[:, co * W2P: (co + tot) * W2P]
                    )
                    nc.gpsimd.dma_gather(
                        msg[:, 0:tot, :],
                        gcat[layer][:],
                        idxl_sb[:, co * 8: (co + tot) * 8],
                        K, K, P,
                        single_packet=False,
                    )
                    jo = 0
                    for p, tp in zip(qd, tots):
                        ps = agg_chunks_at(msg, s_sb, jo, tp)
                        jo += tp
                        nc.vector.tensor_copy(out=pair_slice(pairs[p]), in_=ps[:])

            def layer_agg(layer, xout, bias, hooks=None, post_tile=None):
                # pass 1: remote block-A chunks -> partial (+= for pairs that
                # had a local pass-0, copy otherwise); one gather per quad
                for qd in quads:
                    tots = [int(sched[p, 0]) for p in qd]
                    tot = sum(tots)
                    K = tot * P
                    co = int(chunk_off[qd[0], 0])
                    msg = msgp.tile([P, maxch, P], TABLE_DT, tag="msg", name="msg")
                    s_sb = smatp.tile([P, maxch * W2P], TABLE_DT, tag="smat", name="ssb")
                    nc.sync.dma_start(
                        s_sb[:, 0: tot * W2P], smat_in[:, co * W2P: (co + tot) * W2P]
                    )
                    nc.gpsimd.dma_gather(
                        msg[:, 0:tot, :],
                        g_full[layer][0][:],
                        idx_sb[:, co * 8: (co + tot) * 8],
                        K, K, P,
                        single_packet=False,
                    )
                    jo = 0
                    for p, tp in zip(qd, tots):
                        ps = agg_chunks_at(msg, s_sb, jo, tp)
                        jo += tp
                        if p < LOCAL_PAIRS:
                            nc.vector.tensor_tensor(
                                out=pair_slice(pairs[p]), in0=pair_slice(pairs[p]),
                                in1=ps[:], op=mybir.AluOpType.add,
                            )
                        else:
                            nc.vector.tensor_copy(out=pair_slice(pairs[p]), in_=ps[:])
                # pass 2 (REVERSED quad order): remote block-B + self-loops,
                # add partial, relu
                for rq, qd in enumerate(reversed(quads)):
                    if hooks and rq in hooks:
                        hooks[rq]()
                    tots = [int(sched[p, 1]) for p in qd]
                    tot = sum(tots)
                    K = tot * P
                    co = int(chunk_off[qd[0], 1])
                    msg = msgp.tile([P, maxch, P], TABLE_DT, tag="msg", name="msg")
                    s_sb = smatp.tile([P, maxch * W2P], TABLE_DT, tag="smat", name="ssb")
                    nc.sync.dma_start(
                        s_sb[:, 0: tot * W2P], smat_in[:, co * W2P: (co + tot) * W2P]
                    )
                    nc.gpsimd.dma_gather(
                        msg[:, 0:tot, :],
                        g_full[layer][1][:],
                        idx_sb[:, co * 8: (co + tot) * 8],
                        K, K, P,
                        single_packet=False,
                    )
                    jo = 0
                    for p, tp in zip(qd, tots):
                        pr = pairs[p]
                        ps = psagg.tile([P, W2P], f32, tag="ps_agg", name="psagg")
                        for j in range(tp):
                            nc.tensor.matmul(
                                ps[:], lhsT=msg[:, jo + j, :],
                                rhs=s_sb[:, (jo + j) * W2P: (jo + j + 1) * W2P],
                                start=(j == 0), stop=False,
                            )
                        jo += tp
                        for k, t in enumerate(pr):
                            diag = diagp.tile([P, P], TABLE_DT, tag="diag", name="diag")
                            nc.vector.tensor_scalar(
                                diag[:], ident[:], dinv[:, t: t + 1], None,
                                mybir.AluOpType.mult,
                            )
                            hh, ii = loc_tile(t)
                            nc.tensor.matmul(
                                ps[:, k * P: (k + 1) * P],
                                lhsT=gstage[hh][:, bass.ts(ii, P)], rhs=diag[:],
                                start=(tp == 0), stop=True,
                            )
                        if len(pr) == 1 and tp > 0:
                            # close the accumulation on the unused upper half
                            nc.tensor.matmul(
                                ps[:, P:W2P], lhsT=msg[:, jo - 1, :],
                                rhs=s_sb[:, (jo - 1) * W2P + P: jo * W2P],
                                start=False, stop=True,
                            )
                        nc.vector.tensor_tensor(
                            out=ps[:], in0=ps[:], in1=pair_slice(pr),
                            op=mybir.AluOpType.add,
                        )
                        for k, t in enumerate(pr):
                            hh, ii = loc_tile(t)
                            nc.scalar.activation(
                                xout[hh][:, bass.ts(ii, P)], ps[:, k * P: (k + 1) * P],
                                mybir.ActivationFunctionType.Relu, bias=bias[:],
                            )
                            if post_tile is not None:
                                post_tile(t)

            xT_stages = [xT[:, 0:BLKA], xT[:, BLKA:NPAD]]

            class _W:
                def __init__(self, aps):
                    self.aps = aps

                def __getitem__(self, h):
                    return self.aps[h]

            phase_g_block(_W(xT_stages), w1, 0, 0)
            phase_g_block(_W(xT_stages), w1, 0, 1)
            pass_local(0)
            # B-half tiles (25..48) are pairs 12..24 = reversed quads 0..6;
            # hook the layer-2 B-table production right after them.
            layer_agg(
                0, x1T, b1,
                hooks={7: lambda: phase_g_block(x1T, w2, 1, 1)},
            )
            phase_g_block(x1T, w2, 1, 0)
            pass_local(1)

            def final_tile(t):
                hh, ii = loc_tile(t)
                ps = psphase.tile([P, P], f32, tag="ps_phase", name="psph")
                nc.tensor.matmul(
                    ps[:], lhsT=x1T[hh][:, bass.ts(ii, P)], rhs=lin1[:],
                    start=True, stop=False,
                )
                nc.tensor.matmul(
                    ps[:], lhsT=x2T[hh][:, bass.ts(ii, P)], rhs=lin2[:],
                    start=False, stop=True,
                )
                ot = otilep.tile([P, P], f32, tag="otile", name="otile")
                nc.vector.tensor_tensor(
                    out=ot[:], in0=ps[:], in1=linb[:], op=mybir.AluOpType.add
                )
                nc.sync.dma_start(out_v[:, t, :], ot[:])

            layer_agg(1, x2T, b2, post_tile=final_tile)

    nc.compile()
    return nc


def _in_maps(plan, per_core, W1, b1, W2, b2, lin_W, lin_b):
    D = plan["D"]
    maps = []
    for c in range(plan["C"]):
        pc = per_core[c]
        maps.append(
            {
                "xT": pc["xT"],
                "w1": np.asarray(W1, np.float32),
                "w2": np.asarray(W2, np.float32).astype(TABLE_NP),
                "lin1": np.ascontiguousarray(np.asarray(lin_W, np.float32)[:D]).astype(TABLE_NP),
                "lin2": np.ascontiguousarray(np.asarray(lin_W, np.float32)[D:]).astype(TABLE_NP),
                "b1": np.asarray(b1, np.float32)[:, None],
                "b2": np.asarray(b2, np.float32)[:, None],
                "linb": np.tile(np.asarray(lin_b, np.float32), (P, 1)),
                "dinv": pc["dinv"],
                "idx": pc["idx"],
                "smat": pc["smat"].astype(TABLE_NP),
                "idxl": pc["idxl"],
                "smatl": pc["smatl"].astype(TABLE_NP),
            }
        )
    return maps


def kernel(x, edge_index, W1, b1, W2, b2, lin_W, lin_b):
    x = np.asarray(x, np.float32)
    edge_index = np.asarray(edge_index)

    plan, per_core = _preprocess(x, edge_index)
    nc = _build(plan)
    maps = _in_maps(plan, per_core, W1, b1, W2, b2, lin_W, lin_b)

    N, D, C, NPC = plan["N"], plan["D"], plan["C"], plan["NPC"]
    last_err = None
    for _attempt in range(3):
        try:
            res = run_bass_kernel_spmd(nc, maps, list(range(C)))
            break
        except Exception as e:  # transient NRT device wedges happen
            last_err = e
    else:
        raise last_err

    out = np.empty((N, D), np.float32)
    for c in range(C):
        out[c * NPC: (c + 1) * NPC] = res.results[c]["out"][:NPC]
    return out
